# revision 10
# baseline (speedup 1.0000x reference)
"""LSTM encoder kernel for Trainium2 (Bass/Tile), data-parallel over batch on 8 cores.

Math (per core, batch shard B=256):
  z_t   = Wx @ x_t + 2*Whh @ hh_{t-1} + b          (gates pre-activation, [128, B])
  with g-gate rows of Wx/Whh/b additionally scaled by 2 so that a single
  sigmoid over all 128 gate rows yields  S_g = sigmoid(2 z_g)  and
  tanh(z_g) = 2 S_g - 1.
  Reparametrize cc = c/2, hh = h/2:
    u  = (S_g - 1/2) * S_i          = (i*g)/2
    v  = S_f * cc                   = (f*c)/2
    cc = v + u                      = c_new/2
    S_c = sigmoid(4*cc)             = sigmoid(2*c_new)
    hh = (S_c - 1/2) * S_o          = o*tanh(c_new)/2 = h/2
  Host multiplies the stored hh history by 2 to recover h.

Layouts: gates on partitions (128), batch on free dim. x is staged host-side as
[T, IN, B] per core; output is [T, H, B] (host transposes back).
"""

import numpy as np
from contextlib import ExitStack

import concourse.bass as bass
import concourse.tile as tile
from concourse import bacc, mybir
from concourse.bass_utils import run_bass_kernel_spmd

T_FULL = 512
B_FULL = 2048
IN = 10
H = 32
G = 4 * H  # 128 gate rows
NCORES = 8
B = B_FULL // NCORES  # 256 batch per core

NB = 2          # batch sub-blocks per core (latency pipelining)
FD = B // NB    # free-dim per block
TC = 32         # timesteps per SBUF chunk

DT = mybir.dt.float32
F32R = mybir.dt.float32r
SIG = mybir.ActivationFunctionType.Sigmoid
MULT = mybir.AluOpType.mult
ADD = mybir.AluOpType.add
SUB = mybir.AluOpType.subtract

_CACHE = {}


def _build(t_total=T_FULL, tc=TC, nb=NB):
    """Build the Bass program (one NeuronCore; run SPMD on 8)."""
    fd = B // nb
    nchunk = t_total // tc
    nc = bacc.Bacc(trn_type="TRN2", debug=False, target_bir_lowering=False)

    xT = nc.dram_tensor("xT", [t_total, IN, B], DT, kind="ExternalInput").ap()
    wx = nc.dram_tensor("wx", [IN, G], DT, kind="ExternalInput").ap()
    wh = nc.dram_tensor("wh", [H, G], DT, kind="ExternalInput").ap()
    bg = nc.dram_tensor("bg", [G, 1], DT, kind="ExternalInput").ap()
    hout = nc.dram_tensor("hout", [t_total, H, B], DT, kind="ExternalOutput").ap()

    with tile.TileContext(nc) as tc_, ExitStack() as ctx:
        const = ctx.enter_context(tc_.tile_pool(name="const", bufs=1))
        xpool = ctx.enter_context(tc_.tile_pool(name="xpool", bufs=2))
        hpool = ctx.enter_context(tc_.tile_pool(name="hpool", bufs=2))
        spool = ctx.enter_context(tc_.tile_pool(name="spool", bufs=3))
        cpool = ctx.enter_context(tc_.tile_pool(name="cpool", bufs=2))
        tpool = ctx.enter_context(tc_.tile_pool(name="tpool", bufs=4))
        pspool = ctx.enter_context(tc_.tile_pool(name="pspool", bufs=4, space="PSUM"))

        wx_t = const.tile([IN, G], DT)
        nc.sync.dma_start(wx_t[:], wx)
        wh_t = const.tile([H, G], DT)
        nc.sync.dma_start(wh_t[:], wh)
        bg_t = const.tile([G, 1], DT)
        nc.sync.dma_start(bg_t[:], bg)
        hzero = const.tile([H, B], DT)
        nc.vector.memset(hzero[:], 0.0)

        c_prev = []
        h_prev = []
        for blk in range(nb):
            # cell state kept at partition start 32 (pairs with f = S[32:64])
            c0 = cpool.tile([2 * H, fd], DT, name=f"cc{blk}", tag=f"cc{blk}")
            nc.vector.memset(c0[H:2 * H], 0.0)
            c_prev.append(c0)
            h_prev.append(hzero[:, blk * fd:(blk + 1) * fd])

        for ch in range(nchunk):
            xch = xpool.tile([IN, tc * B], DT, name="xch")
            nc.sync.dma_start(
                xch[:].rearrange("p (t b) -> p t b", t=tc),
                xT[ch * tc:(ch + 1) * tc].rearrange("t p b -> p t b"),
            )
            hch = hpool.tile([H, tc * B], DT, name="hch")
            for s in range(tc):
                ps = []
                for blk in range(nb):
                    col = s * B + blk * fd
                    p = pspool.tile([G, fd], mybir.dt.float32, name="gates")
                    nc.tensor.matmul(
                        p[:], wx_t[:], xch[:, col:col + fd],
                        start=True, stop=False,
                    )
                    nc.tensor.matmul(
                        p[:], wh_t[:], h_prev[blk],
                        start=False, stop=True,
                    )
                    ps.append(p)
                sv = []
                for blk in range(nb):
                    s_t = spool.tile([G, fd], DT, name="sgm")
                    nc.scalar.activation(s_t[:], ps[blk][:], SIG, bias=bg_t[:])
                    sv.append(s_t)
                t1v = []
                for blk in range(nb):
                    # t1 = S_g - 0.5, relocated from start 64 to start 0
                    t1 = tpool.tile([H, fd], DT, name="t1")
                    nc.vector.tensor_scalar(t1[:], sv[blk][2 * H:3 * H],
                                            0.5, None, SUB)
                    t1v.append(t1)
                vv = []
                for blk in range(nb):
                    # v = f * c_prev on GPSIMD (offload), all at start 32
                    v = tpool.tile([2 * H, fd], DT, name="v")
                    nc.gpsimd.tensor_tensor(
                        v[H:2 * H], sv[blk][H:2 * H], c_prev[blk][H:2 * H], MULT)
                    vv.append(v)
                uv = []
                for blk in range(nb):
                    # u = t1 * i  (both at start 0), placed at start 32
                    u = tpool.tile([2 * H, fd], DT, name="u")
                    nc.vector.tensor_tensor(u[H:2 * H], t1v[blk][:],
                                            sv[blk][0:H], MULT)
                    uv.append(u)
                cn = []
                for blk in range(nb):
                    c_new = cpool.tile([2 * H, fd], DT, name=f"ccn{blk}",
                                       tag=f"cc{blk}")
                    nc.vector.tensor_tensor(c_new[H:2 * H], vv[blk][H:2 * H],
                                            uv[blk][H:2 * H], ADD)
                    c_prev[blk] = c_new
                    cn.append(c_new)
                scv = []
                for blk in range(nb):
                    # sc = sigmoid(4*c~), relocated to start 96 (pairs with o)
                    sc = spool.tile([G, fd], DT, name="sc", tag="sc")
                    nc.scalar.activation(sc[3 * H:4 * H], cn[blk][H:2 * H],
                                         SIG, scale=4.0)
                    scv.append(sc)
                for blk in range(nb):
                    col = s * B + blk * fd
                    hdst = hch[:, col:col + fd]
                    nc.vector.scalar_tensor_tensor(
                        hdst, scv[blk][3 * H:4 * H], 0.5,
                        sv[blk][3 * H:4 * H], SUB, MULT)
                    h_prev[blk] = hdst
            nc.sync.dma_start(
                hout[ch * tc:(ch + 1) * tc].rearrange("t p b -> p t b"),
                hch[:].rearrange("p (t b) -> p t b", t=tc),
            )
    nc.compile()
    return nc


def _prep_weights(W_emb, b_emb, W_ih, W_hh, b_ih, b_hh):
    f8 = lambda a: np.asarray(a, np.float64)
    Wx = f8(W_ih) @ f8(W_emb)                                  # [G, IN]
    bgv = f8(W_ih) @ f8(b_emb) + f8(b_ih) + f8(b_hh)           # [G]
    wxT = np.ascontiguousarray(Wx.T)                           # [IN, G]
    whT = np.ascontiguousarray(2.0 * f8(W_hh).T)               # [H, G]
    wxT[:, 2 * H:3 * H] *= 2.0
    whT[:, 2 * H:3 * H] *= 2.0
    bgv = bgv.copy()
    bgv[2 * H:3 * H] *= 2.0
    return (wxT.astype(np.float32), whT.astype(np.float32),
            np.ascontiguousarray(bgv.astype(np.float32).reshape(G, 1)))


def _run(x, W_emb, b_emb, W_ih, W_hh, b_ih, b_hh, trace=False):
    t_total = x.shape[0]
    key = (t_total, TC, NB)
    if key not in _CACHE:
        _CACHE[key] = _build(t_total, TC, NB)
    nc = _CACHE[key]

    wxT, whT, bgv = _prep_weights(W_emb, b_emb, W_ih, W_hh, b_ih, b_hh)
    x = np.asarray(x, np.float32)
    in_maps = []
    for c in range(NCORES):
        xs = np.ascontiguousarray(
            x[:, c * B:(c + 1) * B, :].transpose(0, 2, 1))  # [T, IN, B]
        in_maps.append({"xT": xs, "wx": wxT, "wh": whT, "bg": bgv})

    res = run_bass_kernel_spmd(nc, in_maps, list(range(NCORES)), trace=trace)
    out = np.empty((t_total, B_FULL, H), np.float32)
    for c in range(NCORES):
        out[:, c * B:(c + 1) * B, :] = (
            res.results[c]["hout"].transpose(0, 2, 1) * np.float32(2.0))
    return out, res


def kernel(x, W_emb, b_emb, W_ih, W_hh, b_ih, b_hh):
    out, _ = _run(x, W_emb, b_emb, W_ih, W_hh, b_ih, b_hh, trace=False)
    return out
